# revision 8
# baseline (speedup 1.0000x reference)
"""Trainium2 Bass kernel for NeatModule forward (gnn_message_passing).

Strategy (8 NeuronCores, SPMD):
  - Full batch (128) everywhere; each topo layer's destination nodes are
    sharded across the 8 cores (406 nodes per core per layer).
  - Node state lives in HBM as [20000 nodes, 128 batch] fp32.
  - Per layer, each core gathers the state rows of its edges' source nodes
    with one indirect DMA per ~32-tile chunk (SWDGE row gather, 512B rows),
    landing as msgs tiles [128 edges, 128 batch] in SBUF.
  - The weighted segment-sum over edges is a sequence of PE matmuls:
    host-built A tiles [128 edges, 32 dst] (one-hot by destination, scaled
    by the edge weight) are the stationary operand; each matmul accumulates
    into a 32-row window of a PSUM block, using tile_position col-groups.
  - Activations (sigmoid/tanh/relu by act_id) are applied per PSUM block on
    the Scalar engine, combined with mask-predicated copies on Vector.
  - Each core's 406 computed rows are AllGather'ed and written back into
    the HBM state, which unblocks the next layer's gathers.

Host-side prep prunes edges to the output-reachable subgraph, shards and
packs them into the tile/window structure (identical instruction stream on
all cores; per-core data differs, zero-padded to shared tile counts).
"""
import numpy as np

import concourse.bass as bass
import concourse.mybir as mybir
import concourse.tile as tile
from concourse.vector_clock import ScopedClock, VectorClock
from concourse.tile_rust import add_dep_helper
from concourse.bass_utils import run_bass_kernel_spmd

NUM_INPUTS = 512
NUM_OUTPUTS = 256
NUM_NODES = 20000
NUM_LAYERS = 6
CHUNK = (NUM_NODES - NUM_INPUTS) // NUM_LAYERS  # 3248
NCORES = 8
SLICE = CHUNK // NCORES                         # 406
WPL = (SLICE + 31) // 32                        # 13 windows of 32 dst / layer
NBLK = (WPL + 3) // 4                           # 4 PSUM blocks / layer
P = 128
GCH = 32                                        # gather chunk, in 128-edge tiles

# ---------------------------------------------------------------- tile fixes


def _drain_and_barrier(self, tick_clock, wait_clock):
    # This walrus build rejects instructions carrying many sync waits; emit
    # one nop per proc instead of a single drain waiting on everything.
    gc = tick_clock.global_clock
    n = len(gc)
    for p in range(n):
        t = gc[p]
        if t > 0:
            nop = self.nc.sync.nop(nofuse=True)
            vec = [0] * n
            vec[p] = t
            wait_clock.add_sem_waits(nop.ins, ScopedClock({None: VectorClock(vec)}))
    self.nc.sync.drain()
    self.nc.all_engine_barrier()
    assert self.sems is not None
    popped = self.nc._tile_sem_poison_stack.pop()
    assert popped is self._sem_poison
    self.nc.clear_and_free_semaphores(list(self.sems.allocated().values()))
    self.nc.all_engine_barrier()


tile.TileContext._drain_and_barrier = _drain_and_barrier


def split_waits(nc, K=1):
    """Hoist overflow sync waits onto InstNoOps inserted just before, same engine."""
    n_split = 0
    for f in nc.m.functions:
        for bb in f.blocks:
            insts = list(bb.instructions)
            out = []
            changed = False
            for inst in insts:
                si = inst.sync_info
                if si is not None and si.on_wait is not None and len(si.on_wait) > K:
                    waits = list(si.on_wait)
                    over, keep = waits[:-K], waits[-K:]
                    for j in range(0, len(over), K):
                        out.append(mybir.InstNoOp(
                            name=f"I-waitsplit-{nc.next_id()}",
                            engine=inst.engine,
                            sync_info=mybir.SyncInfo(on_wait=over[j:j + K], on_update=[]),
                        ))
                    si.on_wait = keep
                    inst.sync_info = si
                    changed = True
                    n_split += 1
                out.append(inst)
            if changed:
                bb.instructions = out
    return n_split


# ------------------------------------------------------------------ host prep


def _prune_edges(edge_src, edge_dst):
    """Keep only edges that transitively feed the last NUM_OUTPUTS nodes."""
    needed = np.zeros(NUM_NODES, dtype=bool)
    needed[NUM_NODES - NUM_OUTPUTS:] = True
    layer_of = np.full(NUM_NODES, -1)
    for l in range(NUM_LAYERS):
        layer_of[NUM_INPUTS + l * CHUNK: NUM_INPUTS + (l + 1) * CHUNK] = l
    dst_l = layer_of[edge_dst]
    keep = np.zeros(len(edge_src), dtype=bool)
    for l in range(NUM_LAYERS - 1, -1, -1):
        m = (dst_l == l) & needed[edge_dst]
        keep |= m
        needed[edge_src[m]] = True
    return keep


def prep(weights, edge_src, edge_dst, act_id):
    es = np.asarray(edge_src).astype(np.int64)
    ed = np.asarray(edge_dst).astype(np.int64)
    ew = np.asarray(weights).astype(np.float32)
    act_id = np.asarray(act_id).astype(np.int64)

    keep = _prune_edges(es, ed)
    es, ed, ew = es[keep], ed[keep], ew[keep]

    # dst node -> (core, window, col): round-robin over cores for balance
    rel = (ed - NUM_INPUTS) % CHUNK          # position within its layer
    e_l = (ed - NUM_INPUTS) // CHUNK
    e_nc = rel % NCORES                      # owning core (round-robin)
    srel = rel // NCORES                     # slot within core slice
    e_win = srel // 32
    e_col = srel % 32

    # tiles per (layer, window): shared across cores (max). Empty windows get
    # zero tiles; their dst rows are pruned nodes nobody reads.
    counts = np.zeros((NCORES, NUM_LAYERS, WPL), dtype=np.int64)
    np.add.at(counts, (e_nc, e_l, e_win), 1)
    tiles_lw = -(-counts.max(axis=0) // P)

    tile_off = np.zeros((NUM_LAYERS, WPL), dtype=np.int64)
    off = 0
    for l in range(NUM_LAYERS):
        for w in range(WPL):
            tile_off[l, w] = off
            off += tiles_lw[l, w]
    T_total = int(off)
    T_layer = tiles_lw.sum(axis=1).astype(int)

    src_idx = np.zeros((NCORES, P, T_total), dtype=np.int32)
    A = np.zeros((NCORES, P, 32 * T_total), dtype=np.float32)

    for i in range(NCORES):
        m = e_nc == i
        s_i, w_i = es[m], ew[m]
        l_i, win_i, col_i = e_l[m], e_win[m], e_col[m]
        order = np.lexsort((col_i, win_i, l_i))
        s_i, w_i, l_i, win_i, col_i = (a[order] for a in (s_i, w_i, l_i, win_i, col_i))
        key = l_i * WPL + win_i
        g0 = np.searchsorted(key, np.arange(NUM_LAYERS * WPL), side="left")
        g1 = np.searchsorted(key, np.arange(NUM_LAYERS * WPL), side="right")
        for l in range(NUM_LAYERS):
            for w in range(WPL):
                a0, a1 = g0[l * WPL + w], g1[l * WPL + w]
                if a1 == a0:
                    continue
                slots = np.arange(a1 - a0)
                t = tile_off[l, w] + slots // P
                pp = slots % P
                src_idx[i, pp, t] = s_i[a0:a1]
                A[i, pp, 32 * t + col_i[a0:a1]] = w_i[a0:a1]

    # activation masks per (core, layer, block): mask over the 128 PSUM rows
    msig = np.zeros((NCORES, P, NUM_LAYERS * NBLK), dtype=np.int8)
    mtnh = np.zeros((NCORES, P, NUM_LAYERS * NBLK), dtype=np.int8)
    for i in range(NCORES):
        for l in range(NUM_LAYERS):
            base = NUM_INPUTS + l * CHUNK
            for b in range(NBLK):
                rows = np.arange(128)
                slot = 128 * b + rows            # dst slot within core slice
                valid = slot < SLICE
                node = base + NCORES * slot[valid] + i
                col = l * NBLK + b
                msig[i, rows[valid], col] = (act_id[node] == 0)
                mtnh[i, rows[valid], col] = (act_id[node] == 1)

    meta = dict(T_total=T_total, tiles_lw=tiles_lw, tile_off=tile_off, T_layer=T_layer)
    data = dict(src_idx=src_idx, A=A, msig=msig, mtnh=mtnh)
    return meta, data


# -------------------------------------------------------------- kernel build


def build_nc(meta, reps=1):
    T_total = meta["T_total"]
    tiles_lw = meta["tiles_lw"]
    tile_off = meta["tile_off"]
    T_layer = meta["T_layer"]
    f32 = mybir.dt.float32

    nc = bass.Bass()
    xn = nc.declare_dram_parameter("xn", [NUM_INPUTS, P], f32, isOutput=False)
    sidx = nc.declare_dram_parameter("sidx", [P, T_total], mybir.dt.int32, isOutput=False)
    A_in = nc.declare_dram_parameter("A_in", [P, 32 * T_total], f32, isOutput=False)
    msig_in = nc.declare_dram_parameter("msig", [P, NUM_LAYERS * NBLK], mybir.dt.int8, isOutput=False)
    mtnh_in = nc.declare_dram_parameter("mtnh", [P, NUM_LAYERS * NBLK], mybir.dt.int8, isOutput=False)
    out = nc.declare_dram_parameter("out", [NUM_OUTPUTS, P], f32, isOutput=True)

    state = nc.dram_tensor("state", [NUM_NODES, P], f32)
    contrib = nc.dram_tensor("contrib", [SLICE, P], f32)
    gathered = nc.dram_tensor("gathered", [CHUNK, P], f32, addr_space="Shared")

    AF = mybir.ActivationFunctionType

    with tile.TileContext(nc) as tc:
        with (
            tc.tile_pool(name="big", bufs=1) as big,
            tc.tile_pool(name="msgs", bufs=3) as msgsp,
            tc.tile_pool(name="stage", bufs=2) as stage,
            tc.tile_pool(name="ps", bufs=4, space="PSUM") as psp,
        ):
            A_sb = big.tile([P, 32 * T_total], f32, name="A_sb")
            idx_sb = big.tile([P, T_total], mybir.dt.int32, name="idx_sb")
            msig_sb = big.tile([P, NUM_LAYERS * NBLK], mybir.dt.int8, name="msig_sb")
            mtnh_sb = big.tile([P, NUM_LAYERS * NBLK], mybir.dt.int8, name="mtnh_sb")
            nc.sync.dma_start(A_sb[:], A_in[:])
            nc.sync.dma_start(idx_sb[:], sidx[:])
            nc.sync.dma_start(msig_sb[:], msig_in[:])
            nc.sync.dma_start(mtnh_sb[:], mtnh_in[:])
            x_dma = nc.sync.dma_start(state[:NUM_INPUTS, :], xn[:])

            prev_sync = x_dma          # instruction whose completion gates next layer's gathers
            prev_coll = None           # last collective (for contrib WAR)
            for r in range(reps):
                for l in range(NUM_LAYERS):
                    toff = int(tile_off[l, 0])
                    Tl = int(T_layer[l])
                    nch = -(-Tl // GCH)
                    mts = []
                    for c in range(nch):
                        cn = min(GCH, Tl - c * GCH)
                        mt = msgsp.tile([P, GCH * P], f32, name=f"m_{r}_{l}_{c}", tag="msgs")
                        for tt in range(cn):
                            g = nc.gpsimd.indirect_dma_start(
                                out=mt[:, tt * P:(tt + 1) * P],
                                out_offset=None,
                                in_=state[:, :],
                                in_offset=bass.IndirectOffsetOnAxis(
                                    ap=idx_sb[:, toff + c * GCH + tt: toff + c * GCH + tt + 1],
                                    axis=0),
                            )
                            add_dep_helper(g.ins, prev_sync.ins, reason="state RAW")
                        mts.append(mt)

                    dmas = []
                    for b in range(NBLK):
                        ps = psp.tile([P, P], f32, name=f"ps_{r}_{l}_{b}", tag="ps")
                        wlo, whi = 4 * b, min(4 * b + 4, WPL)
                        blk_tiles = int(tiles_lw[l, wlo:whi].sum())
                        if blk_tiles == 0:
                            # all-empty block (pruned region): rows must still be
                            # defined in case an unpruned zero-in-degree node lands here
                            nc.vector.memset(ps[:], 0.0)
                        for w in range(wlo, whi):
                            j = w % 4
                            nt = int(tiles_lw[l, w])
                            t0 = int(tile_off[l, w])
                            for k in range(nt):
                                gt = t0 + k
                                lt = gt - toff
                                c, o = lt // GCH, lt % GCH
                                nc.tensor.matmul(
                                    ps[32 * j:32 * (j + 1), :],
                                    lhsT=A_sb[:, 32 * gt:32 * (gt + 1)],
                                    rhs=mts[c][:, o * P:(o + 1) * P],
                                    start=(k == 0), stop=(k == nt - 1),
                                    tile_position=(0, 32 * j),
                                )
                        res = stage.tile([P, P], f32, name=f"res_{r}_{l}_{b}", tag="res")
                        sig = stage.tile([P, P], f32, name=f"sig_{r}_{l}_{b}", tag="sig")
                        tnh = stage.tile([P, P], f32, name=f"tnh_{r}_{l}_{b}", tag="tnh")
                        col = l * NBLK + b
                        nc.scalar.activation(res[:], ps[:], AF.Relu)
                        nc.scalar.activation(sig[:], ps[:], AF.Sigmoid)
                        nc.scalar.activation(tnh[:], ps[:], AF.Tanh)
                        nc.vector.copy_predicated(
                            res[:], msig_sb[:, col:col + 1].to_broadcast([P, P]), sig[:])
                        nc.vector.copy_predicated(
                            res[:], mtnh_sb[:, col:col + 1].to_broadcast([P, P]), tnh[:])
                        rows = min(P, SLICE - 128 * b)
                        d = nc.sync.dma_start(contrib[128 * b:128 * b + rows, :], res[:rows, :])
                        if prev_coll is not None:
                            add_dep_helper(d.ins, prev_coll.ins, reason="contrib WAR")
                        dmas.append(d)

                    coll = nc.gpsimd.collective_compute(
                        "AllGather", mybir.AluOpType.bypass,
                        replica_groups=[list(range(NCORES))],
                        ins=[contrib[:]], outs=[gathered[:]],
                    )
                    for d in dmas:
                        add_dep_helper(coll.ins, d.ins, reason="contrib RAW")
                    add_dep_helper(coll.ins, prev_sync.ins, reason="gathered WAR vs prev state copy")
                    base = NUM_INPUTS + l * CHUNK
                    # gathered row (i*SLICE + s) holds node rel = NCORES*s + i
                    st = nc.sync.dma_start(
                        state[base:base + CHUNK, :].rearrange("(s i) b -> i s b", i=NCORES),
                        gathered[:].rearrange("(i s) b -> i s b", i=NCORES),
                    )
                    add_dep_helper(st.ins, coll.ins, reason="gathered RAW")
                    prev_sync = st
                    prev_coll = coll

            od = nc.sync.dma_start(out[:], state[NUM_NODES - NUM_OUTPUTS:, :])
            add_dep_helper(od.ins, prev_sync.ins, reason="out RAW")

    split_waits(nc)
    return nc


# ---------------------------------------------------------------- entry point

_CACHE = {}


def _get_compiled(meta, reps=1):
    key = (meta["T_total"], tuple(meta["T_layer"]), reps)
    if key not in _CACHE:
        _CACHE[key] = build_nc(meta, reps=reps)
    return _CACHE[key]


def kernel(x, weights, edge_src, edge_dst, act_id, layer_masks, steps=1, _reps=1):
    x = np.asarray(x, dtype=np.float32)
    meta, data = prep(weights, edge_src, edge_dst, act_id)
    nc = _get_compiled(meta, reps=_reps)
    xn = np.ascontiguousarray(x.T)  # [512 nodes, 128 batch]
    in_maps = [
        {
            "xn": xn,
            "sidx": data["src_idx"][i],
            "A_in": data["A"][i],
            "msig": data["msig"][i],
            "mtnh": data["mtnh"][i],
        }
        for i in range(NCORES)
    ]
    res = run_bass_kernel_spmd(nc, in_maps, list(range(NCORES)))
    return np.ascontiguousarray(res.results[0]["out"].T)  # [128, 256]


# revision 10
# speedup vs baseline: 2.6203x; 2.6203x over previous
"""Trainium2 Bass kernel for NeatModule forward (gnn_message_passing).

Strategy (8 NeuronCores, SPMD):
  - Full batch (128) everywhere; each topo layer's destination nodes are
    sharded across the 8 cores (406 nodes per core per layer).
  - Node state lives in HBM as [20000 nodes, 128 batch] fp32.
  - Per layer, each core gathers the state rows of its edges' source nodes
    with one indirect DMA per ~32-tile chunk (SWDGE row gather, 512B rows),
    landing as msgs tiles [128 edges, 128 batch] in SBUF.
  - The weighted segment-sum over edges is a sequence of PE matmuls:
    host-built A tiles [128 edges, 32 dst] (one-hot by destination, scaled
    by the edge weight) are the stationary operand; each matmul accumulates
    into a 32-row window of a PSUM block, using tile_position col-groups.
  - Activations (sigmoid/tanh/relu by act_id) are applied per PSUM block on
    the Scalar engine, combined with mask-predicated copies on Vector.
  - Each core's 406 computed rows are AllGather'ed and written back into
    the HBM state, which unblocks the next layer's gathers.

Host-side prep prunes edges to the output-reachable subgraph, shards and
packs them into the tile/window structure (identical instruction stream on
all cores; per-core data differs, zero-padded to shared tile counts).
"""
import numpy as np

import concourse.bass as bass
import concourse.mybir as mybir
import concourse.tile as tile
from concourse.vector_clock import ScopedClock, VectorClock
from concourse.tile_rust import add_dep_helper
from concourse.bass_utils import run_bass_kernel_spmd

NUM_INPUTS = 512
NUM_OUTPUTS = 256
NUM_NODES = 20000
NUM_LAYERS = 6
CHUNK = (NUM_NODES - NUM_INPUTS) // NUM_LAYERS  # 3248
NCORES = 8
SLICE = CHUNK // NCORES                         # 406
WPL = (SLICE + 31) // 32                        # 13 windows of 32 dst / layer
NBLK = (WPL + 3) // 4                           # 4 PSUM blocks / layer
P = 128
GCH = 32                                        # gather chunk, in 128-edge tiles

# ---------------------------------------------------------------- tile fixes


def _drain_and_barrier(self, tick_clock, wait_clock):
    # This walrus build rejects instructions carrying many sync waits; emit
    # one nop per proc instead of a single drain waiting on everything.
    gc = tick_clock.global_clock
    n = len(gc)
    for p in range(n):
        t = gc[p]
        if t > 0:
            nop = self.nc.sync.nop(nofuse=True)
            vec = [0] * n
            vec[p] = t
            wait_clock.add_sem_waits(nop.ins, ScopedClock({None: VectorClock(vec)}))
    self.nc.sync.drain()
    self.nc.all_engine_barrier()
    assert self.sems is not None
    popped = self.nc._tile_sem_poison_stack.pop()
    assert popped is self._sem_poison
    self.nc.clear_and_free_semaphores(list(self.sems.allocated().values()))
    self.nc.all_engine_barrier()


tile.TileContext._drain_and_barrier = _drain_and_barrier


def split_waits(nc, K=1):
    """Hoist overflow sync waits onto InstNoOps inserted just before, same engine."""
    n_split = 0
    for f in nc.m.functions:
        for bb in f.blocks:
            insts = list(bb.instructions)
            out = []
            changed = False
            for inst in insts:
                si = inst.sync_info
                if si is not None and si.on_wait is not None and len(si.on_wait) > K:
                    waits = list(si.on_wait)
                    over, keep = waits[:-K], waits[-K:]
                    for j in range(0, len(over), K):
                        out.append(mybir.InstNoOp(
                            name=f"I-waitsplit-{nc.next_id()}",
                            engine=inst.engine,
                            sync_info=mybir.SyncInfo(on_wait=over[j:j + K], on_update=[]),
                        ))
                    si.on_wait = keep
                    inst.sync_info = si
                    changed = True
                    n_split += 1
                out.append(inst)
            if changed:
                bb.instructions = out
    return n_split


# ------------------------------------------------------------------ host prep


def _prune_edges(edge_src, edge_dst):
    """Keep only edges that transitively feed the last NUM_OUTPUTS nodes."""
    needed = np.zeros(NUM_NODES, dtype=bool)
    needed[NUM_NODES - NUM_OUTPUTS:] = True
    layer_of = np.full(NUM_NODES, -1)
    for l in range(NUM_LAYERS):
        layer_of[NUM_INPUTS + l * CHUNK: NUM_INPUTS + (l + 1) * CHUNK] = l
    dst_l = layer_of[edge_dst]
    keep = np.zeros(len(edge_src), dtype=bool)
    for l in range(NUM_LAYERS - 1, -1, -1):
        m = (dst_l == l) & needed[edge_dst]
        keep |= m
        needed[edge_src[m]] = True
    return keep


def prep(weights, edge_src, edge_dst, act_id):
    es = np.asarray(edge_src).astype(np.int64)
    ed = np.asarray(edge_dst).astype(np.int64)
    ew = np.asarray(weights).astype(np.float32)
    act_id = np.asarray(act_id).astype(np.int64)

    keep = _prune_edges(es, ed)
    es, ed, ew = es[keep], ed[keep], ew[keep]

    # dst node -> (core, window, col): round-robin over cores for balance
    rel = (ed - NUM_INPUTS) % CHUNK          # position within its layer
    e_l = (ed - NUM_INPUTS) // CHUNK
    e_nc = rel % NCORES                      # owning core (round-robin)
    srel = rel // NCORES                     # slot within core slice
    e_win = srel // 32
    e_col = srel % 32

    # tiles per (layer, window): shared across cores (max). Empty windows get
    # zero tiles; their dst rows are pruned nodes nobody reads.
    # Layers 0 and 1 are handled DENSELY (small source ranges) - no gathers.
    counts = np.zeros((NCORES, NUM_LAYERS, WPL), dtype=np.int64)
    np.add.at(counts, (e_nc, e_l, e_win), 1)
    tiles_lw = -(-counts.max(axis=0) // P)
    tiles_lw[0, :] = 0
    tiles_lw[1, :] = 0

    tile_off = np.zeros((NUM_LAYERS, WPL), dtype=np.int64)
    off = 0
    for l in range(NUM_LAYERS):
        for w in range(WPL):
            tile_off[l, w] = off
            off += tiles_lw[l, w]
    T_total = int(off)
    T_layer = tiles_lw.sum(axis=1).astype(int)

    src_idx = np.zeros((NCORES, P, T_total), dtype=np.int32)
    A = np.zeros((NCORES, P, 32 * T_total), dtype=np.float32)

    for i in range(NCORES):
        m = e_nc == i
        s_i, w_i = es[m], ew[m]
        l_i, win_i, col_i = e_l[m], e_win[m], e_col[m]
        order = np.lexsort((col_i, win_i, l_i))
        s_i, w_i, l_i, win_i, col_i = (a[order] for a in (s_i, w_i, l_i, win_i, col_i))
        key = l_i * WPL + win_i
        g0 = np.searchsorted(key, np.arange(NUM_LAYERS * WPL), side="left")
        g1 = np.searchsorted(key, np.arange(NUM_LAYERS * WPL), side="right")
        for l in range(2, NUM_LAYERS):
            for w in range(WPL):
                a0, a1 = g0[l * WPL + w], g1[l * WPL + w]
                if a1 == a0:
                    continue
                slots = np.arange(a1 - a0)
                t = tile_off[l, w] + slots // P
                pp = slots % P
                src_idx[i, pp, t] = s_i[a0:a1]
                A[i, pp, 32 * t + col_i[a0:a1]] = w_i[a0:a1]

    # dense weights for layers 0 (sources < 512) and 1 (sources < 3760)
    DCH = [4, 30]                      # 128-row source chunks per dense layer
    W0 = np.zeros((NCORES, P, DCH[0] * WPL * 32), dtype=np.float32)
    W1 = np.zeros((NCORES, P, DCH[1] * WPL * 32), dtype=np.float32)
    for l, W in ((0, W0), (1, W1)):
        m = e_l == l
        s, wt, i_, wn, cl = es[m], ew[m], e_nc[m], e_win[m], e_col[m]
        ch, pp = s // P, s % P
        np.add.at(W, (i_, pp, (ch * WPL + wn) * 32 + cl), wt)

    # activation masks per (core, layer, block): mask over the 128 PSUM rows
    msig = np.zeros((NCORES, P, NUM_LAYERS * NBLK), dtype=np.int8)
    mtnh = np.zeros((NCORES, P, NUM_LAYERS * NBLK), dtype=np.int8)
    for i in range(NCORES):
        for l in range(NUM_LAYERS):
            base = NUM_INPUTS + l * CHUNK
            for b in range(NBLK):
                rows = np.arange(128)
                slot = 128 * b + rows            # dst slot within core slice
                valid = slot < SLICE
                node = base + NCORES * slot[valid] + i
                col = l * NBLK + b
                msig[i, rows[valid], col] = (act_id[node] == 0)
                mtnh[i, rows[valid], col] = (act_id[node] == 1)

    meta = dict(T_total=T_total, tiles_lw=tiles_lw, tile_off=tile_off, T_layer=T_layer)
    data = dict(src_idx=src_idx, A=A, msig=msig, mtnh=mtnh, W0=W0, W1=W1)
    return meta, data


# -------------------------------------------------------------- kernel build


def build_nc(meta, reps=1):
    T_total = meta["T_total"]
    tiles_lw = meta["tiles_lw"]
    tile_off = meta["tile_off"]
    T_layer = meta["T_layer"]
    f32 = mybir.dt.float32

    nc = bass.Bass()
    xn = nc.declare_dram_parameter("xn", [NUM_INPUTS, P], f32, isOutput=False)
    sidx = nc.declare_dram_parameter("sidx", [P, T_total], mybir.dt.int32, isOutput=False)
    A_in = nc.declare_dram_parameter("A_in", [P, 32 * T_total], f32, isOutput=False)
    msig_in = nc.declare_dram_parameter("msig", [P, NUM_LAYERS * NBLK], mybir.dt.int8, isOutput=False)
    mtnh_in = nc.declare_dram_parameter("mtnh", [P, NUM_LAYERS * NBLK], mybir.dt.int8, isOutput=False)
    DCH = [4, 30]
    W0_in = nc.declare_dram_parameter("W0", [P, DCH[0] * WPL * 32], f32, isOutput=False)
    W1_in = nc.declare_dram_parameter("W1", [P, DCH[1] * WPL * 32], f32, isOutput=False)
    out = nc.declare_dram_parameter("out", [NUM_OUTPUTS, P], f32, isOutput=True)

    state = nc.dram_tensor("state", [NUM_NODES, P], f32)
    contrib = nc.dram_tensor("contrib", [SLICE, P], f32)
    gathered = nc.dram_tensor("gathered", [CHUNK, P], f32, addr_space="Shared")

    AF = mybir.ActivationFunctionType

    with tile.TileContext(nc) as tc:
        with (
            tc.tile_pool(name="big", bufs=1) as big,
            tc.tile_pool(name="msgs", bufs=3) as msgsp,
            tc.tile_pool(name="stage", bufs=2) as stage,
            tc.tile_pool(name="ps", bufs=4, space="PSUM") as psp,
        ):
            A_sb = big.tile([P, 32 * T_total], f32, name="A_sb")
            idx_sb = big.tile([P, T_total], mybir.dt.int32, name="idx_sb")
            msig_sb = big.tile([P, NUM_LAYERS * NBLK], mybir.dt.int8, name="msig_sb")
            mtnh_sb = big.tile([P, NUM_LAYERS * NBLK], mybir.dt.int8, name="mtnh_sb")
            W0_sb = big.tile([P, DCH[0] * WPL * 32], f32, name="W0_sb")
            W1_sb = big.tile([P, DCH[1] * WPL * 32], f32, name="W1_sb")
            nc.sync.dma_start(W0_sb[:], W0_in[:])
            nc.sync.dma_start(W1_sb[:], W1_in[:])
            nc.sync.dma_start(A_sb[:], A_in[:])
            nc.sync.dma_start(idx_sb[:], sidx[:])
            nc.sync.dma_start(msig_sb[:], msig_in[:])
            nc.sync.dma_start(mtnh_sb[:], mtnh_in[:])
            x_dma = nc.sync.dma_start(state[:NUM_INPUTS, :], xn[:])

            prev_sync = x_dma          # instruction whose completion gates next layer's gathers
            prev_coll = None           # last collective (for contrib WAR)
            for r in range(reps):
                for l in range(NUM_LAYERS):
                    dense = l < 2
                    if dense:
                        ndch = DCH[l]
                        W_sb = W0_sb if l == 0 else W1_sb
                        # copy the source range of state into SBUF once per layer
                        srcs = stage.tile([P, ndch * P], f32, name=f"srcs_{r}_{l}", tag="srcs")
                        sc = nc.sync.dma_start(
                            srcs[:].rearrange("p (c b) -> p c b", b=P),
                            state[:ndch * P, :].rearrange("(c p) b -> p c b", p=P))
                        add_dep_helper(sc.ins, prev_sync.ins, reason="state RAW dense")
                    else:
                        toff = int(tile_off[l, 0])
                        Tl = int(T_layer[l])
                        nch = -(-Tl // GCH)
                        mts = []
                        for c in range(nch):
                            cn = min(GCH, Tl - c * GCH)
                            mt = msgsp.tile([P, GCH * P], f32, name=f"m_{r}_{l}_{c}", tag="msgs")
                            for tt in range(cn):
                                g = nc.gpsimd.indirect_dma_start(
                                    out=mt[:, tt * P:(tt + 1) * P],
                                    out_offset=None,
                                    in_=state[:, :],
                                    in_offset=bass.IndirectOffsetOnAxis(
                                        ap=idx_sb[:, toff + c * GCH + tt: toff + c * GCH + tt + 1],
                                        axis=0),
                                )
                                add_dep_helper(g.ins, prev_sync.ins, reason="state RAW")
                            mts.append(mt)

                    dmas = []
                    for b in range(NBLK):
                        ps = psp.tile([P, P], f32, name=f"ps_{r}_{l}_{b}", tag="ps")
                        wlo, whi = 4 * b, min(4 * b + 4, WPL)
                        if dense:
                            for w in range(wlo, whi):
                                j = w % 4
                                for c4 in range(DCH[l]):
                                    nc.tensor.matmul(
                                        ps[32 * j:32 * (j + 1), :],
                                        lhsT=W_sb[:, (c4 * WPL + w) * 32:(c4 * WPL + w) * 32 + 32],
                                        rhs=srcs[:, c4 * P:(c4 + 1) * P],
                                        start=(c4 == 0), stop=(c4 == DCH[l] - 1),
                                        tile_position=(0, 32 * j),
                                    )
                        else:
                            blk_tiles = int(tiles_lw[l, wlo:whi].sum())
                            if blk_tiles == 0:
                                # all-empty block (pruned region): rows must still be
                                # defined in case an unpruned zero-in-degree node lands here
                                nc.vector.memset(ps[:], 0.0)
                            for w in range(wlo, whi):
                                j = w % 4
                                nt = int(tiles_lw[l, w])
                                t0 = int(tile_off[l, w])
                                for k in range(nt):
                                    gt = t0 + k
                                    lt = gt - toff
                                    c, o = lt // GCH, lt % GCH
                                    nc.tensor.matmul(
                                        ps[32 * j:32 * (j + 1), :],
                                        lhsT=A_sb[:, 32 * gt:32 * (gt + 1)],
                                        rhs=mts[c][:, o * P:(o + 1) * P],
                                        start=(k == 0), stop=(k == nt - 1),
                                        tile_position=(0, 32 * j),
                                    )
                        res = stage.tile([P, P], f32, name=f"res_{r}_{l}_{b}", tag="res")
                        sig = stage.tile([P, P], f32, name=f"sig_{r}_{l}_{b}", tag="sig")
                        tnh = stage.tile([P, P], f32, name=f"tnh_{r}_{l}_{b}", tag="tnh")
                        col = l * NBLK + b
                        nc.scalar.activation(res[:], ps[:], AF.Relu)
                        nc.scalar.activation(sig[:], ps[:], AF.Sigmoid)
                        nc.scalar.activation(tnh[:], ps[:], AF.Tanh)
                        nc.vector.copy_predicated(
                            res[:], msig_sb[:, col:col + 1].to_broadcast([P, P]), sig[:])
                        nc.vector.copy_predicated(
                            res[:], mtnh_sb[:, col:col + 1].to_broadcast([P, P]), tnh[:])
                        rows = min(P, SLICE - 128 * b)
                        d = nc.sync.dma_start(contrib[128 * b:128 * b + rows, :], res[:rows, :])
                        if prev_coll is not None:
                            add_dep_helper(d.ins, prev_coll.ins, reason="contrib WAR")
                        dmas.append(d)

                    coll = nc.gpsimd.collective_compute(
                        "AllGather", mybir.AluOpType.bypass,
                        replica_groups=[list(range(NCORES))],
                        ins=[contrib[:]], outs=[gathered[:]],
                    )
                    for d in dmas:
                        add_dep_helper(coll.ins, d.ins, reason="contrib RAW")
                    add_dep_helper(coll.ins, prev_sync.ins, reason="gathered WAR vs prev state copy")
                    base = NUM_INPUTS + l * CHUNK
                    # gathered row (i*SLICE + s) holds node rel = NCORES*s + i
                    st = nc.sync.dma_start(
                        state[base:base + CHUNK, :].rearrange("(s i) b -> i s b", i=NCORES),
                        gathered[:].rearrange("(i s) b -> i s b", i=NCORES),
                    )
                    add_dep_helper(st.ins, coll.ins, reason="gathered RAW")
                    prev_sync = st
                    prev_coll = coll

            od = nc.sync.dma_start(out[:], state[NUM_NODES - NUM_OUTPUTS:, :])
            add_dep_helper(od.ins, prev_sync.ins, reason="out RAW")

    split_waits(nc)
    return nc


# ---------------------------------------------------------------- entry point

_CACHE = {}


def _get_compiled(meta, reps=1):
    key = (meta["T_total"], tuple(meta["T_layer"]), reps)
    if key not in _CACHE:
        _CACHE[key] = build_nc(meta, reps=reps)
    return _CACHE[key]


def kernel(x, weights, edge_src, edge_dst, act_id, layer_masks, steps=1, _reps=1):
    x = np.asarray(x, dtype=np.float32)
    meta, data = prep(weights, edge_src, edge_dst, act_id)
    nc = _get_compiled(meta, reps=_reps)
    xn = np.ascontiguousarray(x.T)  # [512 nodes, 128 batch]
    in_maps = [
        {
            "xn": xn,
            "sidx": data["src_idx"][i],
            "A_in": data["A"][i],
            "msig": data["msig"][i],
            "mtnh": data["mtnh"][i],
            "W0": data["W0"][i],
            "W1": data["W1"][i],
        }
        for i in range(NCORES)
    ]
    res = run_bass_kernel_spmd(nc, in_maps, list(range(NCORES)))
    return np.ascontiguousarray(res.results[0]["out"].T)  # [128, 256]
